# revision 1
# baseline (speedup 1.0000x reference)
"""AtomToPair GNN message-passing kernel for 8 TRN2 NeuronCores.

Math (per molecule, A=64 atoms, F=C=128):
    h0[i,j] = MLP([x_i, x_j]),  h1[i,j] = MLP([x_j, x_i]) = h0[j,i]
    out[i,j] = h0[i,j] + h0[j,i]
so a single MLP pass over all A*A pairs suffices, followed by a
transposed add over the pair grid.

Layer 1 factors per atom: [x_i,x_j]@W0 = x_i@W0top + x_j@W0bot, which we
compute directly on the TensorEngine as two accumulated matmuls whose
moving operand reads xT with broadcast/tiled access patterns (no pair
tensor is ever materialized).

Sharding: data-parallel over batch — each of the 8 cores handles B/8 = 4
molecules with fully replicated weights. All on-chip compute is
feature-major ([C on partitions, pairs on free]); the DRAM output stays
feature-major and the host transposes during the unshard step.
"""

import os
import sys

sys.path.insert(0, "/opt/trn_rl_repo")

import numpy as np

B, A, F, C = 32, 64, 128, 128
NCORES = 8
MPC = B // NCORES          # molecules per core
PAIRS = A * A              # 4096
CHUNK = 512                # pairs per pipeline chunk (= 1 PSUM bank of fp32)
NCHUNK = PAIRS // CHUNK    # 8
IPC = CHUNK // A           # i-values per chunk (8)

_compiled = None


def _build():
    import concourse.bass as bass
    import concourse.tile as tile
    from concourse import bacc, mybir

    fp32 = mybir.dt.float32
    nc = bacc.Bacc("TRN2", target_bir_lowering=False, debug=False,
                   num_devices=NCORES)

    xT = nc.dram_tensor("xT", [F, MPC * A], fp32, kind="ExternalInput").ap()
    w0t = nc.dram_tensor("w0t", [F, C], fp32, kind="ExternalInput").ap()
    w0b = nc.dram_tensor("w0b", [F, C], fp32, kind="ExternalInput").ap()
    w1 = nc.dram_tensor("w1", [C, C], fp32, kind="ExternalInput").ap()
    b0c = nc.dram_tensor("b0c", [C, 1], fp32, kind="ExternalInput").ap()
    b1c = nc.dram_tensor("b1c", [C, 1], fp32, kind="ExternalInput").ap()
    out = nc.dram_tensor("out", [C, MPC * PAIRS], fp32,
                         kind="ExternalOutput").ap()

    Relu = mybir.ActivationFunctionType.Relu
    add_op = mybir.AluOpType.add
    max_op = mybir.AluOpType.max

    with tile.TileContext(nc) as tc:
        with (
            tc.tile_pool(name="const", bufs=1) as const_pool,
            tc.tile_pool(name="y1", bufs=3) as y1_pool,
            tc.tile_pool(name="hrelu", bufs=2) as h_pool,
            tc.tile_pool(name="obuf", bufs=2) as o_pool,
            tc.tile_pool(name="psY", bufs=2, space="PSUM") as psY_pool,
            tc.tile_pool(name="psH", bufs=2, space="PSUM") as psH_pool,
        ):
            xb = const_pool.tile([F, MPC * A], fp32, tag="xb")
            nc.sync.dma_start(xb[:], xT[:])
            w0t_s = const_pool.tile([F, C], fp32, tag="w0t")
            nc.sync.dma_start(w0t_s[:], w0t[:])
            w0b_s = const_pool.tile([F, C], fp32, tag="w0b")
            nc.sync.dma_start(w0b_s[:], w0b[:])
            w1_s = const_pool.tile([C, C], fp32, tag="w1")
            nc.sync.dma_start(w1_s[:], w1[:])
            b0_s = const_pool.tile([C, 1], fp32, tag="b0")
            nc.sync.dma_start(b0_s[:], b0c[:])
            b1_s = const_pool.tile([C, 1], fp32, tag="b1")
            nc.sync.dma_start(b1_s[:], b1c[:])

            for m in range(MPC):
                hr = h_pool.tile([C, PAIRS], fp32, tag="hr")
                for k in range(NCHUNK):
                    # layer 1: Y1pre[:, (i,j)] = W0top.T x_i + W0bot.T x_j
                    psy = psY_pool.tile([C, CHUNK], fp32, tag="psy")
                    xi = xb[:, m * A + k * IPC: m * A + (k + 1) * IPC]
                    rhs_i = xi.unsqueeze(2).to_broadcast((F, IPC, A))
                    xj = xb[:, m * A: (m + 1) * A]
                    rhs_j = xj.unsqueeze(1).to_broadcast((F, IPC, A))
                    ps3 = psy[:].rearrange("c (i j) -> c i j", i=IPC)
                    nc.tensor.matmul(ps3, w0t_s[:], rhs_i,
                                     start=True, stop=False)
                    nc.tensor.matmul(ps3, w0b_s[:], rhs_j,
                                     start=False, stop=True)

                    # relu1 + b0: PSUM -> SBUF (alternate ACT / DVE)
                    y1t = y1_pool.tile([C, CHUNK], fp32, tag="y1t")
                    if k % 2 == 0:
                        nc.scalar.activation(y1t[:], psy[:], Relu,
                                             bias=b0_s[:])
                    else:
                        nc.vector.tensor_scalar(y1t[:], psy[:], b0_s[:],
                                                0.0, add_op, max_op)

                    # layer 2
                    psh = psH_pool.tile([C, CHUNK], fp32, tag="psh")
                    nc.tensor.matmul(psh[:], w1_s[:], y1t[:],
                                     start=True, stop=True)

                    # relu2 + b1: PSUM -> SBUF (opposite engine parity)
                    hslice = hr[:, k * CHUNK: (k + 1) * CHUNK]
                    if k % 2 == 1:
                        nc.scalar.activation(hslice, psh[:], Relu,
                                             bias=b1_s[:])
                    else:
                        nc.vector.tensor_scalar(hslice, psh[:], b1_s[:],
                                                0.0, add_op, max_op)

                # out[:, (i,j)] = H[:, (i,j)] + H[:, (j,i)]
                ot = o_pool.tile([C, PAIRS], fp32, tag="ot")
                h3 = hr[:].rearrange("c (i j) -> c i j", i=A)
                h3m = h3.transpose([0, 2, 1])
                o3 = ot[:].rearrange("c (i j) -> c i j", i=A)
                half = A // 2
                for e in range(2):
                    sl = slice(e * half, (e + 1) * half)
                    nc.vector.tensor_tensor(
                        o3[:, sl], h3[:, sl], h3m[:, sl], add_op)
                    nc.sync.dma_start(
                        out[:, m * PAIRS + e * half * A:
                               m * PAIRS + (e + 1) * half * A],
                        ot[:, e * half * A: (e + 1) * half * A])
    nc.compile()
    return nc


def _get_compiled():
    global _compiled
    if _compiled is None:
        _compiled = _build()
    return _compiled


def kernel(x, W0, b0, W1, b1):
    from concourse.bass_utils import run_bass_kernel_spmd

    x = np.asarray(x, dtype=np.float32)
    W0 = np.asarray(W0, dtype=np.float32)
    b0 = np.asarray(b0, dtype=np.float32)
    W1 = np.asarray(W1, dtype=np.float32)
    b1 = np.asarray(b1, dtype=np.float32)

    w0t = np.ascontiguousarray(W0[:F])
    w0b = np.ascontiguousarray(W0[F:])
    b0c = np.ascontiguousarray(b0[:, None])
    b1c = np.ascontiguousarray(b1[:, None])

    in_maps = []
    for c in range(NCORES):
        xs = x[c * MPC: (c + 1) * MPC]            # [MPC, A, F]
        xTs = np.ascontiguousarray(
            xs.transpose(2, 0, 1).reshape(F, MPC * A))
        in_maps.append({"xT": xTs, "w0t": w0t, "w0b": w0b, "w1": W1,
                        "b0c": b0c, "b1c": b1c})

    nc = _get_compiled()
    res = run_bass_kernel_spmd(nc, in_maps, core_ids=list(range(NCORES)))

    outs = []
    for c in range(NCORES):
        o = res.results[c]["out"]                  # [C, MPC*PAIRS]
        outs.append(o.reshape(C, MPC, PAIRS).transpose(1, 2, 0))
    return np.ascontiguousarray(
        np.concatenate(outs, axis=0), dtype=np.float32)


# revision 2
# speedup vs baseline: 2.1706x; 2.1706x over previous
"""AtomToPair GNN message-passing kernel for 8 TRN2 NeuronCores.

Math (per molecule, A=64 atoms, F=C=128):
    h0[i,j] = MLP([x_i, x_j]),  h1[i,j] = MLP([x_j, x_i]) = h0[j,i]
    out[i,j] = h0[i,j] + h0[j,i]           (symmetric in i,j!)
so a single MLP pass over all A*A pairs suffices, followed by a
transposed add over the pair grid — and since out is symmetric we only
compute/store the block-upper-triangle (j >= 8*floor(i/8)) and mirror
on the host.

Layer 1 factors per atom: [x_i,x_j]@W0 = x_i@W0top + x_j@W0bot, computed
on the TensorEngine as two accumulated bf16 matmuls whose moving operand
reads xT with broadcast/tiled access patterns (no pair tensor is ever
materialized).  Matmuls run in bf16 (fp32 matmul on TRN2 is the slow
LOW_HIGH two-pass mode); PSUM accumulation stays fp32 and the final
output is fp32.

Sharding: data-parallel over batch — each of the 8 cores handles B/8 = 4
molecules with fully replicated weights. On-chip compute is
feature-major ([C on partitions, pairs on free]); the host transposes
to the reference layout during the unshard step.
"""

import sys

sys.path.insert(0, "/opt/trn_rl_repo")

import numpy as np

B, A, F, C = 32, 64, 128, 128
NCORES = 8
MPC = B // NCORES          # molecules per core
PAIRS = A * A              # 4096
IB = 8                     # i-block (rows per chunk)
NCHUNK = A // IB           # 8 chunks per molecule
# packed block-triangle: chunk k holds rows i in [8k,8k+8), cols j in [8k,64)
TRI_W = [A - IB * k for k in range(NCHUNK)]      # 64,56,...,8
TRI_OFF = [IB * sum(TRI_W[:k]) for k in range(NCHUNK)]
TRI_COLS = IB * sum(TRI_W)                        # 2304 per molecule

_compiled = None


def _build():
    import concourse.bass as bass
    import concourse.tile as tile
    from concourse import bacc, mybir

    fp32 = mybir.dt.float32
    bf16 = mybir.dt.bfloat16
    nc = bacc.Bacc("TRN2", target_bir_lowering=False, debug=False,
                   num_devices=NCORES)

    xT = nc.dram_tensor("xT", [F, MPC * A], bf16, kind="ExternalInput").ap()
    w0t = nc.dram_tensor("w0t", [F, C], bf16, kind="ExternalInput").ap()
    w0b = nc.dram_tensor("w0b", [F, C], bf16, kind="ExternalInput").ap()
    w1 = nc.dram_tensor("w1", [C, C], bf16, kind="ExternalInput").ap()
    b0c = nc.dram_tensor("b0c", [C, 1], fp32, kind="ExternalInput").ap()
    b1c = nc.dram_tensor("b1c", [C, 1], fp32, kind="ExternalInput").ap()
    out = nc.dram_tensor("out", [C, MPC * TRI_COLS], fp32,
                         kind="ExternalOutput").ap()

    Relu = mybir.ActivationFunctionType.Relu
    add_op = mybir.AluOpType.add
    max_op = mybir.AluOpType.max

    with tile.TileContext(nc) as tc:
        with (
            tc.tile_pool(name="const", bufs=1) as const_pool,
            tc.tile_pool(name="y1", bufs=3) as y1_pool,
            tc.tile_pool(name="hrelu", bufs=2) as h_pool,
            tc.tile_pool(name="obuf", bufs=2) as o_pool,
            tc.tile_pool(name="psY", bufs=2, space="PSUM") as psY_pool,
            tc.tile_pool(name="psH", bufs=2, space="PSUM") as psH_pool,
        ):
            xb = const_pool.tile([F, MPC * A], bf16, tag="xb")
            nc.sync.dma_start(xb[:], xT[:])
            w0t_s = const_pool.tile([F, C], bf16, tag="w0t")
            nc.sync.dma_start(w0t_s[:], w0t[:])
            w0b_s = const_pool.tile([F, C], bf16, tag="w0b")
            nc.sync.dma_start(w0b_s[:], w0b[:])
            w1_s = const_pool.tile([C, C], bf16, tag="w1")
            nc.sync.dma_start(w1_s[:], w1[:])
            b0_s = const_pool.tile([C, 1], fp32, tag="b0")
            nc.sync.dma_start(b0_s[:], b0c[:])
            b1_s = const_pool.tile([C, 1], fp32, tag="b1")
            nc.sync.dma_start(b1_s[:], b1c[:])

            for m in range(MPC):
                hr = h_pool.tile([C, PAIRS], fp32, tag="hr")
                xm = xb[:, m * A: (m + 1) * A]
                # two chunks (2*IB i-values = 1024 pairs) per pipeline step
                for q in range(NCHUNK // 2):
                    psy = psY_pool.tile([C, 2 * IB * A], fp32, tag="psy")
                    for h in range(2):
                        k = 2 * q + h
                        xi = xm[:, k * IB: (k + 1) * IB]
                        rhs_i = xi.unsqueeze(2).to_broadcast((F, IB, A))
                        rhs_j = xm.unsqueeze(1).to_broadcast((F, IB, A))
                        ps3 = psy[:, h * IB * A: (h + 1) * IB * A].rearrange(
                            "c (i j) -> c i j", i=IB)
                        nc.tensor.matmul(ps3, w0t_s[:], rhs_i,
                                         start=True, stop=False)
                        nc.tensor.matmul(ps3, w0b_s[:], rhs_j,
                                         start=False, stop=True)

                    # relu1 + b0 -> bf16 Y1T   (PSUM -> SBUF)
                    y1t = y1_pool.tile([C, 2 * IB * A], bf16, tag="y1t")
                    if q != 3:
                        nc.scalar.activation(y1t[:], psy[:], Relu,
                                             bias=b0_s[:])
                    else:
                        nc.vector.tensor_scalar(y1t[:], psy[:], b0_s[:],
                                                0.0, add_op, max_op)

                    # layer 2
                    psh = psH_pool.tile([C, 2 * IB * A], fp32, tag="psh")
                    nc.tensor.matmul(psh[:, :IB * A], w1_s[:],
                                     y1t[:, :IB * A], start=True, stop=True)
                    nc.tensor.matmul(psh[:, IB * A:], w1_s[:],
                                     y1t[:, IB * A:], start=True, stop=True)

                    # relu2 + b1 -> fp32 H    (PSUM -> SBUF)
                    hslice = hr[:, q * 2 * IB * A: (q + 1) * 2 * IB * A]
                    if q != 1:
                        nc.scalar.activation(hslice, psh[:], Relu,
                                             bias=b1_s[:])
                    else:
                        nc.vector.tensor_scalar(hslice, psh[:], b1_s[:],
                                                0.0, add_op, max_op)

                # block-triangle mirror add:
                # ot[:, k-block] = H[i,j] + H[j,i],  i in [8k,8k+8), j>=8k
                ot = o_pool.tile([C, TRI_COLS], fp32, tag="ot")
                h3 = hr[:].rearrange("c (i j) -> c i j", i=A)
                for k in range(NCHUNK):
                    w = TRI_W[k]
                    straight = h3[:, k * IB: (k + 1) * IB, k * IB:]
                    mirror = h3[:, k * IB:, k * IB: (k + 1) * IB]
                    mirror = mirror.transpose([0, 2, 1])
                    o3 = ot[:, TRI_OFF[k]: TRI_OFF[k] + IB * w].rearrange(
                        "c (i j) -> c i j", i=IB)
                    nc.vector.tensor_tensor(o3, straight, mirror, add_op)
                nc.sync.dma_start(
                    out[:, m * TRI_COLS: (m + 1) * TRI_COLS], ot[:])
    nc.compile()
    return nc


def _get_compiled():
    global _compiled
    if _compiled is None:
        _compiled = _build()
    return _compiled


def _shard_inputs(x, W0, b0, W1, b1):
    import ml_dtypes

    bf = ml_dtypes.bfloat16
    w0t = np.ascontiguousarray(W0[:F]).astype(bf)
    w0b = np.ascontiguousarray(W0[F:]).astype(bf)
    w1b = np.ascontiguousarray(W1).astype(bf)
    b0c = np.ascontiguousarray(b0[:, None]).astype(np.float32)
    b1c = np.ascontiguousarray(b1[:, None]).astype(np.float32)
    in_maps = []
    for c in range(NCORES):
        xs = x[c * MPC: (c + 1) * MPC]            # [MPC, A, F]
        xTs = np.ascontiguousarray(
            xs.transpose(2, 0, 1).reshape(F, MPC * A)).astype(bf)
        in_maps.append({"xT": xTs, "w0t": w0t, "w0b": w0b, "w1": w1b,
                        "b0c": b0c, "b1c": b1c})
    return in_maps


def _unshard(results):
    """[C, MPC*TRI_COLS] per core -> full (B, A*A, C) with mirror fill."""
    full = np.empty((B, A, A, C), dtype=np.float32)
    for c in range(NCORES):
        o = results[c]["out"]                     # [C, MPC*TRI_COLS]
        for m in range(MPC):
            bidx = c * MPC + m
            pk = o[:, m * TRI_COLS: (m + 1) * TRI_COLS]
            for k in range(NCHUNK):
                w = TRI_W[k]
                blk = pk[:, TRI_OFF[k]: TRI_OFF[k] + IB * w]
                blk = blk.reshape(C, IB, w).transpose(1, 2, 0)
                full[bidx, k * IB: (k + 1) * IB, k * IB:] = blk
                if k > 0:
                    # mirror: rows i in this block, cols j < 8k come from
                    # the transposed computed blocks (j,i)
                    full[bidx, k * IB: (k + 1) * IB, : k * IB] = \
                        full[bidx, : k * IB, k * IB: (k + 1) * IB] \
                        .transpose(1, 0, 2)
    return full.reshape(B, A * A, C)


def kernel(x, W0, b0, W1, b1):
    from concourse.bass_utils import run_bass_kernel_spmd

    x = np.asarray(x, dtype=np.float32)
    W0 = np.asarray(W0, dtype=np.float32)
    b0 = np.asarray(b0, dtype=np.float32)
    W1 = np.asarray(W1, dtype=np.float32)
    b1 = np.asarray(b1, dtype=np.float32)

    in_maps = _shard_inputs(x, W0, b0, W1, b1)
    nc = _get_compiled()
    res = run_bass_kernel_spmd(nc, in_maps, core_ids=list(range(NCORES)))
    return _unshard(res.results)
